# revision 21
# baseline (speedup 1.0000x reference)
"""Trainium2 Bass kernel for nn_Loss2_53996328845453 (segment_reduce).

Computes a multi-term image loss over B=16 samples of 512x512 images:
  total = 10*L_exp + 1*L_tv + 10*L_color + 50*L_sem

Strategy (pure data parallel, B sharded 2-per-core across 8 cores):
  - Memory-bound problem -> minimize HBM bytes: the host prebuilds both
    gram operand streams in fp8 e3m4 (values are uniform [0,1), e3m4
    keeps 4 mantissa bits; squares are computed exactly in f64 and
    quantized once):
      W stream (stationary): [M0..7, M0^2..7^2] chunk-major
        [P, slab, F, 16] -> contiguous 16KB/partition DMA runs and
        contiguous 128-col weight APs (FWL-eligible fp8).
      X stream (moving):     [R(3), R^2(3), 1, I(3)] lane-major
        [P, slab, 10, F].
    The device does ZERO reshuffles/squares for the gram path: 8 chunks
    per matmul, lhsT 128 weight cols, rhs 80 moving cols, fp32 PSUM
    accumulation; only chunk-diagonal entries are meaningful and the
    host extracts them.
  - I channel sums: ones-vector [P,1] stationary x I moving lanes,
    Q=128 chunks per matmul -> psum [1, 384], host folds.
  - Exposure: PE with block-ones weights [128,8] sums 16-row groups;
    [8,2048] partial dumped to host which does the 16-col sums.
  - TV: vertical in-band diffs via bidiagonal shift matmul (PE, exact);
    horizontal diffs via one DVE subtract; both abs-reduced to scalars
    on GPSIMD (XYZWC) after ACT evacuation of PSUM. Band-boundary row
    pairs (6 rows of 512 per sample) are computed on host.
  - Final scalar assembly on host in float64 from tiny per-core outputs.
"""
import os
import sys

import numpy as np

try:
    import concourse.bacc as bacc  # noqa: F401
except ImportError:
    sys.path.insert(0, "/opt/trn_rl_repo")

from contextlib import ExitStack

import ml_dtypes
import concourse.bacc as bacc
import concourse.tile as tile
from concourse import mybir
from concourse import bass_utils

# problem constants (hardcoded per spec)
B, NCORES = 16, 8
BLOC = B // NCORES            # 2 samples per core
H = W = 512
HW = H * W                    # 262144 px
K, C = 8, 3
P = 128                       # SBUF partitions / matmul contraction
FALL = HW // P                # 2048 chunks of 128 px per sample
FS = 512                      # chunks per slab
NSLAB = FALL // FS            # 4
Q = 16                        # chunks packed per gram matmul
QI = 128                      # chunks per I-sum matmul
XL = 10                       # x lanes: R(3), ones, R^2(3), I(3)
XA = C + 1                    # moving lanes vs M weights: R, ones
XB = 2 * C + 1                # moving lanes vs M^2 weights: R, ones, R^2
E_EXP = 0.6
PATCH = 16
L_EXP_W, L_TV_W, L_COLOR_W, L_SEM_W = 10.0, 1.0, 10.0, 50.0

f32 = mybir.dt.float32
f16 = mybir.dt.float16
f8 = mybir.dt.float8e3
np_f8 = ml_dtypes.float8_e3m4

_NC_CACHE = {}
LAST_RESULTS = None


def _build_nc():
    nc = bacc.Bacc("TRN2")
    W_d = nc.dram_tensor(
        "W_loc", [BLOC, P, NSLAB, FS, K], f8, kind="ExternalInput"
    )
    X_d = nc.dram_tensor(
        "X_loc", [BLOC, P, NSLAB, XL, FS], f8, kind="ExternalInput"
    )
    L_d = nc.dram_tensor("L_loc", [BLOC, 1, H, W], f16, kind="ExternalInput")
    S_d = nc.dram_tensor("shift_d", [P, P], f16, kind="ExternalInput")
    We_d = nc.dram_tensor("wexp_d", [P, K], f16, kind="ExternalInput")
    On_d = nc.dram_tensor("ones_d", [P, 1], f8, kind="ExternalInput")

    gram_o = nc.dram_tensor(
        "gram_o", [BLOC, P, (XA + XB) * Q], f32, kind="ExternalOutput"
    )
    io_o = nc.dram_tensor("io_o", [BLOC, 1, C * QI], f32, kind="ExternalOutput")
    expo_o = nc.dram_tensor("expo_o", [BLOC, K, 4 * W], f32, kind="ExternalOutput")
    # TV partials: cols 0:4 vertical (per band), 4:8 horizontal
    lout_o = nc.dram_tensor("lout_o", [BLOC, P, 8], f32, kind="ExternalOutput")

    with ExitStack() as ctx:
        tc = ctx.enter_context(tile.TileContext(nc))
        wp = ctx.enter_context(tc.tile_pool(name="wp", bufs=4))
        wq = ctx.enter_context(tc.tile_pool(name="wq", bufs=4))
        xp = ctx.enter_context(tc.tile_pool(name="xp", bufs=4))
        lp = ctx.enter_context(tc.tile_pool(name="lp", bufs=2))
        sp = ctx.enter_context(tc.tile_pool(name="sp", bufs=2))
        op = ctx.enter_context(tc.tile_pool(name="op", bufs=2))
        cs = ctx.enter_context(tc.tile_pool(name="cs", bufs=1))
        pm = ctx.enter_context(tc.tile_pool(name="pm", bufs=2, space="PSUM"))
        pm2 = ctx.enter_context(tc.tile_pool(name="pm2", bufs=2, space="PSUM"))
        pi = ctx.enter_context(tc.tile_pool(name="pi", bufs=2, space="PSUM"))
        ve = ctx.enter_context(tc.tile_pool(name="ve", bufs=1, space="PSUM"))
        vp = ctx.enter_context(tc.tile_pool(name="vp", bufs=1, space="PSUM"))

        Ssb = cs.tile([P, P], f16)
        nc.gpsimd.dma_start(out=Ssb, in_=S_d[:])
        WexpSB = cs.tile([P, K], f16)
        nc.gpsimd.dma_start(out=WexpSB, in_=We_d[:])
        OnesSB = cs.tile([P, 1], f8)
        nc.gpsimd.dma_start(out=OnesSB, in_=On_d[:])

        for b in range(BLOC):
            psum_m = pm.tile([P, XA * Q], f32, tag="psum_m")
            psum_m2 = pm2.tile([P, XB * Q], f32, tag="psum_m2")
            psum_i = pi.tile([1, C * QI], f32, tag="psum_i")

            # ---- L path first: early PE work while gram slabs stream in
            Lb = L_d[b, 0]  # [512, 512]
            Lbands = Lb.rearrange("(r p) w -> p r w", p=P)          # [128,4,512]
            Lt = lp.tile([P, 4, W], f16, tag="Lt")
            nc.gpsimd.dma_start(out=Lt, in_=Lbands)

            expo_sb = op.tile([K, 4 * W], f32, tag="expo_sb")
            ot = op.tile([P, 8], f32, tag="ot")
            for r in range(4):
                # exposure 16-row-group sums on PE
                psum_e = ve.tile([K, W], f32, tag="psum_e")
                nc.tensor.matmul(
                    psum_e, lhsT=WexpSB, rhs=Lt[:, r, :], start=True, stop=True
                )
                nc.scalar.copy(expo_sb[:, r * W : (r + 1) * W], psum_e)
                # vertical TV diffs on PE (exact +-1 weights), DVE abs-reduce
                psum_v = vp.tile([P, W], f32, tag="psum_v")
                nc.tensor.matmul(
                    psum_v, lhsT=Ssb, rhs=Lt[:, r, :], start=True, stop=True
                )
                nc.vector.tensor_reduce(
                    ot[:, r : r + 1],
                    psum_v,
                    axis=mybir.AxisListType.X,
                    op=mybir.AluOpType.add,
                    apply_absolute_value=True,
                )
            nc.gpsimd.dma_start(out=expo_o[b], in_=expo_sb)
            # horizontal TV: DVE subtract + abs-reduce
            dh = sp.tile([P, 4, W], f16, tag="dh")
            nc.vector.tensor_sub(
                dh[:, :, 0 : W - 1], Lt[:, :, 1:W], Lt[:, :, 0 : W - 1]
            )
            nc.vector.tensor_reduce(
                ot[:, 4:8],
                dh[:, :, 0 : W - 1],
                axis=mybir.AxisListType.X,
                op=mybir.AluOpType.add,
                apply_absolute_value=True,
            )
            nc.gpsimd.dma_start(out=lout_o[b], in_=ot)

            # ---- gram slabs: M chunk-major from host, M^2 squared on-chip
            for s in range(NSLAB):
                Wt = wp.tile([P, FS, K], f8, tag="Wt")
                nc.sync.dma_start(out=Wt, in_=W_d[b, :, s])
                W2 = wq.tile([P, FS, K], f8, tag="W2")
                nc.vector.tensor_mul(
                    W2[:, 0 : FS // 2, :], Wt[:, 0 : FS // 2, :],
                    Wt[:, 0 : FS // 2, :],
                )
                nc.scalar.activation(
                    W2[:, FS // 2 :, :], Wt[:, FS // 2 :, :],
                    mybir.ActivationFunctionType.Square,
                )
                Xt = xp.tile([P, XL, FS], f8, tag="Xt")
                nc.scalar.dma_start(out=Xt, in_=X_d[b, :, s])

                for m in range(FS // Q):
                    j = m * Q
                    g = s * (FS // Q) + m
                    nc.tensor.matmul(
                        psum_m,
                        lhsT=Wt[:, j : j + Q, :],
                        rhs=Xt[:, 0:XA, j : j + Q],
                        start=(g == 0),
                        stop=(g == FALL // Q - 1),
                    )
                    nc.tensor.matmul(
                        psum_m2,
                        lhsT=W2[:, j : j + Q, :],
                        rhs=Xt[:, 0:XB, j : j + Q],
                        start=(g == 0),
                        stop=(g == FALL // Q - 1),
                    )
                for m in range(FS // QI):
                    j = m * QI
                    g = s * (FS // QI) + m
                    nc.tensor.matmul(
                        psum_i,
                        lhsT=OnesSB,
                        rhs=Xt[:, 2 * C + 1 : XL, j : j + QI],
                        start=(g == 0),
                        stop=(g == FALL // QI - 1),
                    )

            # ---- evacuate gram + I sums
            gram_sb = op.tile([P, (XA + XB) * Q], f32, tag="gram_sb")
            nc.scalar.copy(gram_sb[:, 0 : XA * Q], psum_m)
            nc.scalar.copy(gram_sb[:, XA * Q :], psum_m2)
            nc.gpsimd.dma_start(out=gram_o[b], in_=gram_sb)
            io_sb = op.tile([1, C * QI], f32, tag="io_sb")
            nc.scalar.copy(io_sb, psum_i)
            nc.gpsimd.dma_start(out=io_o[b], in_=io_sb)

    nc.finalize()
    return nc


def _get_nc():
    if "nc" not in _NC_CACHE:
        _NC_CACHE["nc"] = _build_nc()
    return _NC_CACHE["nc"]


def kernel(L, R, I_enh, semantic_masks):
    global LAST_RESULTS
    nc = _get_nc()

    # bidiagonal shift matrix: out[m] = L[m+1] - L[m] for m < 127
    S = np.zeros((P, P), dtype=np.float16)
    for m in range(P - 1):
        S[m + 1, m] = 1.0
        S[m, m] = -1.0
    # block-ones weights: row-group j sums partitions 16j..16j+15
    Wexp = np.zeros((P, K), dtype=np.float16)
    for j in range(K):
        Wexp[16 * j : 16 * (j + 1), j] = 1.0
    Ones = np.ones((P, 1), dtype=np_f8)

    R64 = np.asarray(R, dtype=np.float64)
    I64 = np.asarray(I_enh, dtype=np.float64)
    M64 = np.asarray(semantic_masks, dtype=np.float64)

    # W stream: [B, P, NSLAB, FS, 8] = M chunk-major e3m4 (M^2 on-chip)
    Wm = M64.reshape(B, K, P, NSLAB, FS).transpose(0, 2, 3, 4, 1)
    Wfull = Wm.astype(np_f8)
    # X stream: [B, P, NSLAB, 10, FS] = [R, 1, R^2, I] lane-major e3m4
    Xr = R64.reshape(B, C, P, NSLAB, FS).transpose(0, 2, 3, 1, 4)
    Xi = I64.reshape(B, C, P, NSLAB, FS).transpose(0, 2, 3, 1, 4)
    Xfull = np.empty((B, P, NSLAB, XL, FS), dtype=np_f8)
    Xfull[..., 0:C, :] = Xr.astype(np_f8)
    Xfull[..., C, :] = np_f8(1.0)
    Xfull[..., C + 1 : 2 * C + 1, :] = (Xr * Xr).astype(np_f8)
    Xfull[..., 2 * C + 1 : XL, :] = Xi.astype(np_f8)

    in_maps = []
    for i in range(NCORES):
        sl = slice(BLOC * i, BLOC * (i + 1))
        in_maps.append(
            {
                "W_loc": np.ascontiguousarray(Wfull[sl]),
                "X_loc": np.ascontiguousarray(Xfull[sl]),
                "L_loc": np.ascontiguousarray(L[sl], dtype=np.float16),
                "shift_d": S,
                "wexp_d": Wexp,
                "ones_d": Ones,
            }
        )

    res = bass_utils.run_bass_kernel_spmd(
        nc, in_maps, core_ids=list(range(NCORES))
    )
    LAST_RESULTS = res

    # ---- host-side combine in float64
    exp_acc = 0.0
    tv_acc_v = 0.0
    tv_acc_h = 0.0
    col_acc = 0.0
    sem_acc = 0.0
    L64 = np.asarray(L, dtype=np.float64)
    for core in range(NCORES):
        o = res.results[core]
        gram_d = o["gram_o"].astype(np.float64)  # [BLOC, 128, 176]
        io_d = o["io_o"].astype(np.float64)      # [BLOC, 1, 384]
        expo_d = o["expo_o"].astype(np.float64)  # [BLOC, 8, 2048]
        lout = o["lout_o"].astype(np.float64)    # [BLOC, P, 8]
        for b in range(BLOC):
            # gram diag: row = q*8 + l, col = x*16 + q
            Gm = np.einsum(
                "qlxq->lx", gram_d[b, :, 0 : XA * Q].reshape(Q, K, XA, Q)
            )  # [8, 4]: x = R(3), ones
            Gm2 = np.einsum(
                "qlxq->lx", gram_d[b, :, XA * Q :].reshape(Q, K, XB, Q)
            )  # [8, 7]: x = R(3), ones, R^2(3)
            sRM = Gm[:, 0:C].T           # [c, k]
            sRM2 = Gm2[:, 0:C].T
            sR2M2 = Gm2[:, C + 1 : 2 * C + 1].T
            nvec = Gm[:, C] + 1e-6
            sM2 = Gm2[:, C]
            mean = sRM / nvec[None, :]
            var = (sR2M2 - 2.0 * mean * sRM2 + mean * mean * sM2[None, :]).sum(
                axis=0
            ) / nvec
            sem_acc += var.sum()

            # color: io col = lane*128 + chunk-phase
            sumI = io_d[b, 0].reshape(C, QI).sum(axis=1)
            mI = sumI / HW
            col_acc += (
                (mI[0] - mI[1]) ** 2 + (mI[0] - mI[2]) ** 2 + (mI[1] - mI[2]) ** 2
            )

            # exposure: [j, r*512+w] -> patch sums -> (Lp-E)^2
            eo = expo_d[b].reshape(K, 4, 32, PATCH).sum(-1)   # [j, r, wg]
            Lp = eo.transpose(1, 0, 2).reshape(32, 32) / (PATCH * PATCH)
            exp_acc += ((Lp - E_EXP) ** 2).sum()

            tv_acc_v += lout[b, :, 0:4].sum()
            tv_acc_h += lout[b, :, 4:8].sum()

            # band-boundary vertical diffs (3 row pairs) on host
            bsamp = core * BLOC + b
            for r in range(1, 4):
                tv_acc_v += np.abs(
                    L64[bsamp, 0, 128 * r] - L64[bsamp, 0, 128 * r - 1]
                ).sum()

    L_exp = exp_acc / (B * 32 * 32)
    L_tv = tv_acc_v / (B * 1 * (H - 1) * W) + tv_acc_h / (B * 1 * H * (W - 1))
    L_color = col_acc / B
    L_sem = sem_acc / B
    total = (
        L_EXP_W * L_exp + L_TV_W * L_tv + L_COLOR_W * L_color + L_SEM_W * L_sem
    )
    return np.float32(total)


# revision 24
# speedup vs baseline: 1.0333x; 1.0333x over previous
"""Trainium2 Bass kernel for nn_Loss2_53996328845453 (segment_reduce).

Computes a multi-term image loss over B=16 samples of 512x512 images:
  total = 10*L_exp + 1*L_tv + 10*L_color + 50*L_sem

Strategy (pure data parallel, B sharded 2-per-core across 8 cores):
  - Memory-bound problem -> minimize HBM bytes: the host prebuilds both
    gram operand streams in fp8 e3m4 (values are uniform [0,1), e3m4
    keeps 4 mantissa bits; squares are computed exactly in f64 and
    quantized once):
      W stream (stationary): [M0..7, M0^2..7^2] chunk-major
        [P, slab, F, 16] -> contiguous 16KB/partition DMA runs and
        contiguous 128-col weight APs (FWL-eligible fp8).
      X stream (moving):     [R(3), R^2(3), 1, I(3)] lane-major
        [P, slab, 10, F].
    The device does ZERO reshuffles/squares for the gram path: 8 chunks
    per matmul, lhsT 128 weight cols, rhs 80 moving cols, fp32 PSUM
    accumulation; only chunk-diagonal entries are meaningful and the
    host extracts them.
  - I channel sums: ones-vector [P,1] stationary x I moving lanes,
    Q=128 chunks per matmul -> psum [1, 384], host folds.
  - Exposure: PE with block-ones weights [128,8] sums 16-row groups;
    [8,2048] partial dumped to host which does the 16-col sums.
  - TV: vertical in-band diffs via bidiagonal shift matmul (PE, exact);
    horizontal diffs via one DVE subtract; both abs-reduced to scalars
    on GPSIMD (XYZWC) after ACT evacuation of PSUM. Band-boundary row
    pairs (6 rows of 512 per sample) are computed on host.
  - Final scalar assembly on host in float64 from tiny per-core outputs.
"""
import os
import sys

import numpy as np

try:
    import concourse.bacc as bacc  # noqa: F401
except ImportError:
    sys.path.insert(0, "/opt/trn_rl_repo")

from contextlib import ExitStack

import ml_dtypes
import concourse.bacc as bacc
import concourse.tile as tile
from concourse import mybir
from concourse import bass_utils

# problem constants (hardcoded per spec)
B, NCORES = 16, 8
BLOC = B // NCORES            # 2 samples per core
H = W = 512
HW = H * W                    # 262144 px
K, C = 8, 3
P = 128                       # SBUF partitions / matmul contraction
FALL = HW // P                # 2048 chunks of 128 px per sample
FS = 512                      # chunks per slab
NSLAB = FALL // FS            # 4
Q = 16                        # chunks packed per gram matmul
QI = 128                      # chunks per I-sum matmul
XL = 10                       # x lanes: R(3), ones, R^2(3), I(3)
XA = C + 1                    # moving lanes vs M weights: R, ones
XB = 2 * C + 1                # moving lanes vs M^2 weights: R, ones, R^2
E_EXP = 0.6
PATCH = 16
L_EXP_W, L_TV_W, L_COLOR_W, L_SEM_W = 10.0, 1.0, 10.0, 50.0

f32 = mybir.dt.float32
f16 = mybir.dt.float16
f8 = mybir.dt.float8e3
np_f8 = ml_dtypes.float8_e3m4

_NC_CACHE = {}
LAST_RESULTS = None


def _build_nc():
    nc = bacc.Bacc("TRN2")
    W_d = nc.dram_tensor(
        "W_loc", [BLOC, P, NSLAB, FS, K], f8, kind="ExternalInput"
    )
    X_d = nc.dram_tensor(
        "X_loc", [BLOC, P, NSLAB, XL, FS], f8, kind="ExternalInput"
    )
    L_d = nc.dram_tensor("L_loc", [BLOC, 1, H, W], f16, kind="ExternalInput")
    S_d = nc.dram_tensor("shift_d", [P, P], f16, kind="ExternalInput")
    We_d = nc.dram_tensor("wexp_d", [P, K], f16, kind="ExternalInput")
    On_d = nc.dram_tensor("ones_d", [P, 1], f8, kind="ExternalInput")

    gram_o = nc.dram_tensor(
        "gram_o", [BLOC, P, (XA + XB) * Q], f32, kind="ExternalOutput"
    )
    io_o = nc.dram_tensor("io_o", [BLOC, 1, C * QI], f32, kind="ExternalOutput")
    expo_o = nc.dram_tensor("expo_o", [BLOC, K, 4 * W], f32, kind="ExternalOutput")
    # TV partials: cols 0:4 vertical (per band), 4:8 horizontal
    lout_o = nc.dram_tensor("lout_o", [BLOC, P, 8], f32, kind="ExternalOutput")

    with ExitStack() as ctx:
        tc = ctx.enter_context(tile.TileContext(nc))
        wp = ctx.enter_context(tc.tile_pool(name="wp", bufs=5))
        wq = ctx.enter_context(tc.tile_pool(name="wq", bufs=5))
        xp = ctx.enter_context(tc.tile_pool(name="xp", bufs=5))
        lp = ctx.enter_context(tc.tile_pool(name="lp", bufs=2))
        sp = ctx.enter_context(tc.tile_pool(name="sp", bufs=2))
        op = ctx.enter_context(tc.tile_pool(name="op", bufs=2))
        cs = ctx.enter_context(tc.tile_pool(name="cs", bufs=1))
        pm = ctx.enter_context(tc.tile_pool(name="pm", bufs=2, space="PSUM"))
        pm2 = ctx.enter_context(tc.tile_pool(name="pm2", bufs=2, space="PSUM"))
        pi = ctx.enter_context(tc.tile_pool(name="pi", bufs=2, space="PSUM"))
        ve = ctx.enter_context(tc.tile_pool(name="ve", bufs=1, space="PSUM"))
        vp = ctx.enter_context(tc.tile_pool(name="vp", bufs=1, space="PSUM"))

        Ssb = cs.tile([P, P], f16)
        nc.gpsimd.dma_start(out=Ssb, in_=S_d[:])
        WexpSB = cs.tile([P, K], f16)
        nc.gpsimd.dma_start(out=WexpSB, in_=We_d[:])
        OnesSB = cs.tile([P, 1], f8)
        nc.gpsimd.dma_start(out=OnesSB, in_=On_d[:])

        for b in range(BLOC):
            psum_m = pm.tile([P, XA * Q], f32, tag="psum_m")
            psum_m2 = pm2.tile([P, XB * Q], f32, tag="psum_m2")
            psum_i = pi.tile([1, C * QI], f32, tag="psum_i")

            # ---- issue all slab DMAs + M^2 builds first (prefetch depth 5)
            Wts, W2s, Xts = [], [], []
            FD1 = 176  # DVE third / ACT two-thirds: ~balanced at fp8 rates
            for s in range(NSLAB):
                Wt = wp.tile([P, FS, K], f8, tag="Wt")
                nc.sync.dma_start(out=Wt, in_=W_d[b, :, s])
                Xt = xp.tile([P, XL, FS], f8, tag="Xt")
                nc.scalar.dma_start(out=Xt, in_=X_d[b, :, s])
                W2 = wq.tile([P, FS, K], f8, tag="W2")
                nc.vector.tensor_mul(
                    W2[:, 0:FD1, :], Wt[:, 0:FD1, :], Wt[:, 0:FD1, :]
                )
                nc.scalar.activation(
                    W2[:, FD1:, :], Wt[:, FD1:, :],
                    mybir.ActivationFunctionType.Square,
                )
                Wts.append(Wt)
                W2s.append(W2)
                Xts.append(Xt)

            # ---- L path: early PE work while gram slabs stream in
            Lb = L_d[b, 0]  # [512, 512]
            Lbands = Lb.rearrange("(r p) w -> p r w", p=P)          # [128,4,512]
            Lt = lp.tile([P, 4, W], f16, tag="Lt")
            nc.gpsimd.dma_start(out=Lt, in_=Lbands)

            expo_sb = op.tile([K, 4 * W], f32, tag="expo_sb")
            ot = op.tile([P, 8], f32, tag="ot")
            for r in range(4):
                # exposure 16-row-group sums on PE
                psum_e = ve.tile([K, W], f32, tag="psum_e")
                nc.tensor.matmul(
                    psum_e, lhsT=WexpSB, rhs=Lt[:, r, :], start=True, stop=True
                )
                nc.scalar.copy(expo_sb[:, r * W : (r + 1) * W], psum_e)
                # vertical TV diffs on PE (exact +-1 weights), DVE abs-reduce
                psum_v = vp.tile([P, W], f32, tag="psum_v")
                nc.tensor.matmul(
                    psum_v, lhsT=Ssb, rhs=Lt[:, r, :], start=True, stop=True
                )
                nc.vector.tensor_reduce(
                    ot[:, r : r + 1],
                    psum_v,
                    axis=mybir.AxisListType.X,
                    op=mybir.AluOpType.add,
                    apply_absolute_value=True,
                )
            nc.gpsimd.dma_start(out=expo_o[b], in_=expo_sb)
            # horizontal TV: DVE subtract + abs-reduce
            dh = sp.tile([P, 4, W], f16, tag="dh")
            nc.vector.tensor_sub(
                dh[:, :, 0 : W - 1], Lt[:, :, 1:W], Lt[:, :, 0 : W - 1]
            )
            nc.vector.tensor_reduce(
                ot[:, 4:8],
                dh[:, :, 0 : W - 1],
                axis=mybir.AxisListType.X,
                op=mybir.AluOpType.add,
                apply_absolute_value=True,
            )
            nc.gpsimd.dma_start(out=lout_o[b], in_=ot)

            # ---- gram matmuls over the prefetched slabs
            for s in range(NSLAB):
                Wt, W2, Xt = Wts[s], W2s[s], Xts[s]
                for m in range(FS // Q):
                    j = m * Q
                    g = s * (FS // Q) + m
                    nc.tensor.matmul(
                        psum_m,
                        lhsT=Wt[:, j : j + Q, :],
                        rhs=Xt[:, 0:XA, j : j + Q],
                        start=(g == 0),
                        stop=(g == FALL // Q - 1),
                    )
                    nc.tensor.matmul(
                        psum_m2,
                        lhsT=W2[:, j : j + Q, :],
                        rhs=Xt[:, 0:XB, j : j + Q],
                        start=(g == 0),
                        stop=(g == FALL // Q - 1),
                    )
                for m in range(FS // QI):
                    j = m * QI
                    g = s * (FS // QI) + m
                    nc.tensor.matmul(
                        psum_i,
                        lhsT=OnesSB,
                        rhs=Xt[:, 2 * C + 1 : XL, j : j + QI],
                        start=(g == 0),
                        stop=(g == FALL // QI - 1),
                    )

            # ---- evacuate gram + I sums
            gram_sb = op.tile([P, (XA + XB) * Q], f32, tag="gram_sb")
            nc.scalar.copy(gram_sb[:, 0 : XA * Q], psum_m)
            nc.scalar.copy(gram_sb[:, XA * Q :], psum_m2)
            nc.gpsimd.dma_start(out=gram_o[b], in_=gram_sb)
            io_sb = op.tile([1, C * QI], f32, tag="io_sb")
            nc.scalar.copy(io_sb, psum_i)
            nc.gpsimd.dma_start(out=io_o[b], in_=io_sb)

    nc.finalize()
    return nc


def _get_nc():
    if "nc" not in _NC_CACHE:
        _NC_CACHE["nc"] = _build_nc()
    return _NC_CACHE["nc"]


def kernel(L, R, I_enh, semantic_masks):
    global LAST_RESULTS
    nc = _get_nc()

    # bidiagonal shift matrix: out[m] = L[m+1] - L[m] for m < 127
    S = np.zeros((P, P), dtype=np.float16)
    for m in range(P - 1):
        S[m + 1, m] = 1.0
        S[m, m] = -1.0
    # block-ones weights: row-group j sums partitions 16j..16j+15
    Wexp = np.zeros((P, K), dtype=np.float16)
    for j in range(K):
        Wexp[16 * j : 16 * (j + 1), j] = 1.0
    Ones = np.ones((P, 1), dtype=np_f8)

    R64 = np.asarray(R, dtype=np.float64)
    I64 = np.asarray(I_enh, dtype=np.float64)
    M64 = np.asarray(semantic_masks, dtype=np.float64)

    # W stream: [B, P, NSLAB, FS, 8] = M chunk-major e3m4 (M^2 on-chip)
    Wm = M64.reshape(B, K, P, NSLAB, FS).transpose(0, 2, 3, 4, 1)
    Wfull = Wm.astype(np_f8)
    # X stream: [B, P, NSLAB, 10, FS] = [R, 1, R^2, I] lane-major e3m4
    Xr = R64.reshape(B, C, P, NSLAB, FS).transpose(0, 2, 3, 1, 4)
    Xi = I64.reshape(B, C, P, NSLAB, FS).transpose(0, 2, 3, 1, 4)
    Xfull = np.empty((B, P, NSLAB, XL, FS), dtype=np_f8)
    Xfull[..., 0:C, :] = Xr.astype(np_f8)
    Xfull[..., C, :] = np_f8(1.0)
    Xfull[..., C + 1 : 2 * C + 1, :] = (Xr * Xr).astype(np_f8)
    Xfull[..., 2 * C + 1 : XL, :] = Xi.astype(np_f8)

    in_maps = []
    for i in range(NCORES):
        sl = slice(BLOC * i, BLOC * (i + 1))
        in_maps.append(
            {
                "W_loc": np.ascontiguousarray(Wfull[sl]),
                "X_loc": np.ascontiguousarray(Xfull[sl]),
                "L_loc": np.ascontiguousarray(L[sl], dtype=np.float16),
                "shift_d": S,
                "wexp_d": Wexp,
                "ones_d": Ones,
            }
        )

    res = bass_utils.run_bass_kernel_spmd(
        nc, in_maps, core_ids=list(range(NCORES))
    )
    LAST_RESULTS = res

    # ---- host-side combine in float64
    exp_acc = 0.0
    tv_acc_v = 0.0
    tv_acc_h = 0.0
    col_acc = 0.0
    sem_acc = 0.0
    L64 = np.asarray(L, dtype=np.float64)
    for core in range(NCORES):
        o = res.results[core]
        gram_d = o["gram_o"].astype(np.float64)  # [BLOC, 128, 176]
        io_d = o["io_o"].astype(np.float64)      # [BLOC, 1, 384]
        expo_d = o["expo_o"].astype(np.float64)  # [BLOC, 8, 2048]
        lout = o["lout_o"].astype(np.float64)    # [BLOC, P, 8]
        for b in range(BLOC):
            # gram diag: row = q*8 + l, col = x*16 + q
            Gm = np.einsum(
                "qlxq->lx", gram_d[b, :, 0 : XA * Q].reshape(Q, K, XA, Q)
            )  # [8, 4]: x = R(3), ones
            Gm2 = np.einsum(
                "qlxq->lx", gram_d[b, :, XA * Q :].reshape(Q, K, XB, Q)
            )  # [8, 7]: x = R(3), ones, R^2(3)
            sRM = Gm[:, 0:C].T           # [c, k]
            sRM2 = Gm2[:, 0:C].T
            sR2M2 = Gm2[:, C + 1 : 2 * C + 1].T
            nvec = Gm[:, C] + 1e-6
            sM2 = Gm2[:, C]
            mean = sRM / nvec[None, :]
            var = (sR2M2 - 2.0 * mean * sRM2 + mean * mean * sM2[None, :]).sum(
                axis=0
            ) / nvec
            sem_acc += var.sum()

            # color: io col = lane*128 + chunk-phase
            sumI = io_d[b, 0].reshape(C, QI).sum(axis=1)
            mI = sumI / HW
            col_acc += (
                (mI[0] - mI[1]) ** 2 + (mI[0] - mI[2]) ** 2 + (mI[1] - mI[2]) ** 2
            )

            # exposure: [j, r*512+w] -> patch sums -> (Lp-E)^2
            eo = expo_d[b].reshape(K, 4, 32, PATCH).sum(-1)   # [j, r, wg]
            Lp = eo.transpose(1, 0, 2).reshape(32, 32) / (PATCH * PATCH)
            exp_acc += ((Lp - E_EXP) ** 2).sum()

            tv_acc_v += lout[b, :, 0:4].sum()
            tv_acc_h += lout[b, :, 4:8].sum()

            # band-boundary vertical diffs (3 row pairs) on host
            bsamp = core * BLOC + b
            for r in range(1, 4):
                tv_acc_v += np.abs(
                    L64[bsamp, 0, 128 * r] - L64[bsamp, 0, 128 * r - 1]
                ).sum()

    L_exp = exp_acc / (B * 32 * 32)
    L_tv = tv_acc_v / (B * 1 * (H - 1) * W) + tv_acc_h / (B * 1 * H * (W - 1))
    L_color = col_acc / B
    L_sem = sem_acc / B
    total = (
        L_EXP_W * L_exp + L_TV_W * L_tv + L_COLOR_W * L_color + L_SEM_W * L_sem
    )
    return np.float32(total)


# revision 25
# speedup vs baseline: 1.1272x; 1.0909x over previous
"""Trainium2 Bass kernel for nn_Loss2_53996328845453 (segment_reduce).

Computes a multi-term image loss over B=16 samples of 512x512 images:
  total = 10*L_exp + 1*L_tv + 10*L_color + 50*L_sem

Strategy (pure data parallel, B sharded 2-per-core across 8 cores):
  - Memory-bound problem -> minimize HBM bytes: the host prebuilds both
    gram operand streams in fp8 e3m4 (values are uniform [0,1), e3m4
    keeps 4 mantissa bits; squares are computed exactly in f64 and
    quantized once):
      W stream (stationary): [M0..7, M0^2..7^2] chunk-major
        [P, slab, F, 16] -> contiguous 16KB/partition DMA runs and
        contiguous 128-col weight APs (FWL-eligible fp8).
      X stream (moving):     [R(3), R^2(3), 1, I(3)] lane-major
        [P, slab, 10, F].
    The device does ZERO reshuffles/squares for the gram path: 8 chunks
    per matmul, lhsT 128 weight cols, rhs 80 moving cols, fp32 PSUM
    accumulation; only chunk-diagonal entries are meaningful and the
    host extracts them.
  - I channel sums: ones-vector [P,1] stationary x I moving lanes,
    Q=128 chunks per matmul -> psum [1, 384], host folds.
  - Exposure: PE with block-ones weights [128,8] sums 16-row groups;
    [8,2048] partial dumped to host which does the 16-col sums.
  - TV: vertical in-band diffs via bidiagonal shift matmul (PE, exact);
    horizontal diffs via one DVE subtract; both abs-reduced to scalars
    on GPSIMD (XYZWC) after ACT evacuation of PSUM. Band-boundary row
    pairs (6 rows of 512 per sample) are computed on host.
  - Final scalar assembly on host in float64 from tiny per-core outputs.
"""
import os
import sys

import numpy as np

try:
    import concourse.bacc as bacc  # noqa: F401
except ImportError:
    sys.path.insert(0, "/opt/trn_rl_repo")

from contextlib import ExitStack

import ml_dtypes
import concourse.bacc as bacc
import concourse.tile as tile
from concourse import mybir
from concourse import bass_utils

# problem constants (hardcoded per spec)
B, NCORES = 16, 8
BLOC = B // NCORES            # 2 samples per core
H = W = 512
HW = H * W                    # 262144 px
K, C = 8, 3
P = 128                       # SBUF partitions / matmul contraction
FALL = HW // P                # 2048 chunks of 128 px per sample
FS = 512                      # chunks per slab
NSLAB = FALL // FS            # 4
Q = 16                        # chunks packed per gram matmul
QI = 128                      # chunks per I-sum matmul
XL = 10                       # x lanes: R(3), ones, R^2(3), I(3)
XA = C + 1                    # moving lanes vs M weights: R, ones
XB = 2 * C + 1                # moving lanes vs M^2 weights: R, ones, R^2
E_EXP = 0.6
PATCH = 16
L_EXP_W, L_TV_W, L_COLOR_W, L_SEM_W = 10.0, 1.0, 10.0, 50.0

f32 = mybir.dt.float32
f16 = mybir.dt.float16
f8 = mybir.dt.float8e3
np_f8 = ml_dtypes.float8_e3m4

_NC_CACHE = {}
LAST_RESULTS = None


def _build_nc():
    nc = bacc.Bacc("TRN2")
    W_d = nc.dram_tensor(
        "W_loc", [BLOC, P, NSLAB, FS, K], f8, kind="ExternalInput"
    )
    X_d = nc.dram_tensor(
        "X_loc", [BLOC, P, NSLAB, XL, FS], f8, kind="ExternalInput"
    )
    L_d = nc.dram_tensor("L_loc", [BLOC, 1, H, W], f16, kind="ExternalInput")
    S_d = nc.dram_tensor("shift_d", [P, P], f16, kind="ExternalInput")
    We_d = nc.dram_tensor("wexp_d", [P, K], f16, kind="ExternalInput")
    On_d = nc.dram_tensor("ones_d", [P, 1], f8, kind="ExternalInput")

    gram_o = nc.dram_tensor(
        "gram_o", [BLOC, P, (XA + XB) * Q], f32, kind="ExternalOutput"
    )
    io_o = nc.dram_tensor("io_o", [BLOC, 1, C * QI], f32, kind="ExternalOutput")
    expo_o = nc.dram_tensor("expo_o", [BLOC, K, 4 * W], f32, kind="ExternalOutput")
    # TV partials: cols 0:4 vertical (per band), 4:8 horizontal
    lout_o = nc.dram_tensor("lout_o", [BLOC, P, 8], f32, kind="ExternalOutput")

    with ExitStack() as ctx:
        tc = ctx.enter_context(tile.TileContext(nc))
        wp = ctx.enter_context(tc.tile_pool(name="wp", bufs=5))
        wq = ctx.enter_context(tc.tile_pool(name="wq", bufs=5))
        xp = ctx.enter_context(tc.tile_pool(name="xp", bufs=5))
        lp = ctx.enter_context(tc.tile_pool(name="lp", bufs=2))
        sp = ctx.enter_context(tc.tile_pool(name="sp", bufs=2))
        op = ctx.enter_context(tc.tile_pool(name="op", bufs=2))
        cs = ctx.enter_context(tc.tile_pool(name="cs", bufs=1))
        pm = ctx.enter_context(tc.tile_pool(name="pm", bufs=2, space="PSUM"))
        pm2 = ctx.enter_context(tc.tile_pool(name="pm2", bufs=2, space="PSUM"))
        pi = ctx.enter_context(tc.tile_pool(name="pi", bufs=2, space="PSUM"))
        ve = ctx.enter_context(tc.tile_pool(name="ve", bufs=1, space="PSUM"))
        vp = ctx.enter_context(tc.tile_pool(name="vp", bufs=1, space="PSUM"))

        Ssb = cs.tile([P, P], f16)
        nc.gpsimd.dma_start(out=Ssb, in_=S_d[:])
        WexpSB = cs.tile([P, K], f16)
        nc.gpsimd.dma_start(out=WexpSB, in_=We_d[:])
        OnesSB = cs.tile([P, 1], f8)
        nc.gpsimd.dma_start(out=OnesSB, in_=On_d[:])

        for b in range(BLOC):
            psum_m = pm.tile([P, XA * Q], f32, tag="psum_m")
            psum_m2 = pm2.tile([P, XB * Q], f32, tag="psum_m2")
            psum_i = pi.tile([1, C * QI], f32, tag="psum_i")

            # ---- issue all slab DMAs + M^2 builds first (prefetch depth 5)
            Wts, W2s, Xts = [], [], []
            FD1 = 176  # DVE third / ACT two-thirds: ~balanced at fp8 rates
            for s in range(NSLAB):
                Wt = wp.tile([P, FS, K], f8, tag="Wt")
                nc.sync.dma_start(out=Wt, in_=W_d[b, :, s])
                Xt = xp.tile([P, XL, FS], f8, tag="Xt")
                nc.sync.dma_start(out=Xt, in_=X_d[b, :, s])
                W2 = wq.tile([P, FS, K], f8, tag="W2")
                nc.vector.tensor_mul(
                    W2[:, 0:FD1, :], Wt[:, 0:FD1, :], Wt[:, 0:FD1, :]
                )
                nc.scalar.activation(
                    W2[:, FD1:, :], Wt[:, FD1:, :],
                    mybir.ActivationFunctionType.Square,
                )
                Wts.append(Wt)
                W2s.append(W2)
                Xts.append(Xt)

            # ---- L path: early PE work while gram slabs stream in
            Lb = L_d[b, 0]  # [512, 512]
            Lbands = Lb.rearrange("(r p) w -> p r w", p=P)          # [128,4,512]
            Lt = lp.tile([P, 4, W], f16, tag="Lt")
            nc.scalar.dma_start(out=Lt, in_=Lbands)

            expo_sb = op.tile([K, 4 * W], f32, tag="expo_sb")
            ot = op.tile([P, 8], f32, tag="ot")
            for r in range(4):
                # exposure 16-row-group sums on PE
                psum_e = ve.tile([K, W], f32, tag="psum_e")
                nc.tensor.matmul(
                    psum_e, lhsT=WexpSB, rhs=Lt[:, r, :], start=True, stop=True
                )
                nc.scalar.copy(expo_sb[:, r * W : (r + 1) * W], psum_e)
                # vertical TV diffs on PE (exact +-1 weights), DVE abs-reduce
                psum_v = vp.tile([P, W], f32, tag="psum_v")
                nc.tensor.matmul(
                    psum_v, lhsT=Ssb, rhs=Lt[:, r, :], start=True, stop=True
                )
                nc.vector.tensor_reduce(
                    ot[:, r : r + 1],
                    psum_v,
                    axis=mybir.AxisListType.X,
                    op=mybir.AluOpType.add,
                    apply_absolute_value=True,
                )
            nc.sync.dma_start(out=expo_o[b], in_=expo_sb)
            # horizontal TV: DVE subtract + abs-reduce
            dh = sp.tile([P, 4, W], f16, tag="dh")
            nc.vector.tensor_sub(
                dh[:, :, 0 : W - 1], Lt[:, :, 1:W], Lt[:, :, 0 : W - 1]
            )
            nc.vector.tensor_reduce(
                ot[:, 4:8],
                dh[:, :, 0 : W - 1],
                axis=mybir.AxisListType.X,
                op=mybir.AluOpType.add,
                apply_absolute_value=True,
            )
            nc.sync.dma_start(out=lout_o[b], in_=ot)

            # ---- gram matmuls over the prefetched slabs
            for s in range(NSLAB):
                Wt, W2, Xt = Wts[s], W2s[s], Xts[s]
                for m in range(FS // Q):
                    j = m * Q
                    g = s * (FS // Q) + m
                    nc.tensor.matmul(
                        psum_m,
                        lhsT=Wt[:, j : j + Q, :],
                        rhs=Xt[:, 0:XA, j : j + Q],
                        start=(g == 0),
                        stop=(g == FALL // Q - 1),
                    )
                    nc.tensor.matmul(
                        psum_m2,
                        lhsT=W2[:, j : j + Q, :],
                        rhs=Xt[:, 0:XB, j : j + Q],
                        start=(g == 0),
                        stop=(g == FALL // Q - 1),
                    )
                for m in range(FS // QI):
                    j = m * QI
                    g = s * (FS // QI) + m
                    nc.tensor.matmul(
                        psum_i,
                        lhsT=OnesSB,
                        rhs=Xt[:, 2 * C + 1 : XL, j : j + QI],
                        start=(g == 0),
                        stop=(g == FALL // QI - 1),
                    )

            # ---- evacuate gram + I sums
            gram_sb = op.tile([P, (XA + XB) * Q], f32, tag="gram_sb")
            nc.scalar.copy(gram_sb[:, 0 : XA * Q], psum_m)
            nc.scalar.copy(gram_sb[:, XA * Q :], psum_m2)
            nc.sync.dma_start(out=gram_o[b], in_=gram_sb)
            io_sb = op.tile([1, C * QI], f32, tag="io_sb")
            nc.scalar.copy(io_sb, psum_i)
            nc.sync.dma_start(out=io_o[b], in_=io_sb)

    nc.finalize()
    return nc


def _get_nc():
    if "nc" not in _NC_CACHE:
        _NC_CACHE["nc"] = _build_nc()
    return _NC_CACHE["nc"]


def kernel(L, R, I_enh, semantic_masks):
    global LAST_RESULTS
    nc = _get_nc()

    # bidiagonal shift matrix: out[m] = L[m+1] - L[m] for m < 127
    S = np.zeros((P, P), dtype=np.float16)
    for m in range(P - 1):
        S[m + 1, m] = 1.0
        S[m, m] = -1.0
    # block-ones weights: row-group j sums partitions 16j..16j+15
    Wexp = np.zeros((P, K), dtype=np.float16)
    for j in range(K):
        Wexp[16 * j : 16 * (j + 1), j] = 1.0
    Ones = np.ones((P, 1), dtype=np_f8)

    R64 = np.asarray(R, dtype=np.float64)
    I64 = np.asarray(I_enh, dtype=np.float64)
    M64 = np.asarray(semantic_masks, dtype=np.float64)

    # W stream: [B, P, NSLAB, FS, 8] = M chunk-major e3m4 (M^2 on-chip)
    Wm = M64.reshape(B, K, P, NSLAB, FS).transpose(0, 2, 3, 4, 1)
    Wfull = Wm.astype(np_f8)
    # X stream: [B, P, NSLAB, 10, FS] = [R, 1, R^2, I] lane-major e3m4
    Xr = R64.reshape(B, C, P, NSLAB, FS).transpose(0, 2, 3, 1, 4)
    Xi = I64.reshape(B, C, P, NSLAB, FS).transpose(0, 2, 3, 1, 4)
    Xfull = np.empty((B, P, NSLAB, XL, FS), dtype=np_f8)
    Xfull[..., 0:C, :] = Xr.astype(np_f8)
    Xfull[..., C, :] = np_f8(1.0)
    Xfull[..., C + 1 : 2 * C + 1, :] = (Xr * Xr).astype(np_f8)
    Xfull[..., 2 * C + 1 : XL, :] = Xi.astype(np_f8)

    in_maps = []
    for i in range(NCORES):
        sl = slice(BLOC * i, BLOC * (i + 1))
        in_maps.append(
            {
                "W_loc": np.ascontiguousarray(Wfull[sl]),
                "X_loc": np.ascontiguousarray(Xfull[sl]),
                "L_loc": np.ascontiguousarray(L[sl], dtype=np.float16),
                "shift_d": S,
                "wexp_d": Wexp,
                "ones_d": Ones,
            }
        )

    res = bass_utils.run_bass_kernel_spmd(
        nc, in_maps, core_ids=list(range(NCORES))
    )
    LAST_RESULTS = res

    # ---- host-side combine in float64
    exp_acc = 0.0
    tv_acc_v = 0.0
    tv_acc_h = 0.0
    col_acc = 0.0
    sem_acc = 0.0
    L64 = np.asarray(L, dtype=np.float64)
    for core in range(NCORES):
        o = res.results[core]
        gram_d = o["gram_o"].astype(np.float64)  # [BLOC, 128, 176]
        io_d = o["io_o"].astype(np.float64)      # [BLOC, 1, 384]
        expo_d = o["expo_o"].astype(np.float64)  # [BLOC, 8, 2048]
        lout = o["lout_o"].astype(np.float64)    # [BLOC, P, 8]
        for b in range(BLOC):
            # gram diag: row = q*8 + l, col = x*16 + q
            Gm = np.einsum(
                "qlxq->lx", gram_d[b, :, 0 : XA * Q].reshape(Q, K, XA, Q)
            )  # [8, 4]: x = R(3), ones
            Gm2 = np.einsum(
                "qlxq->lx", gram_d[b, :, XA * Q :].reshape(Q, K, XB, Q)
            )  # [8, 7]: x = R(3), ones, R^2(3)
            sRM = Gm[:, 0:C].T           # [c, k]
            sRM2 = Gm2[:, 0:C].T
            sR2M2 = Gm2[:, C + 1 : 2 * C + 1].T
            nvec = Gm[:, C] + 1e-6
            sM2 = Gm2[:, C]
            mean = sRM / nvec[None, :]
            var = (sR2M2 - 2.0 * mean * sRM2 + mean * mean * sM2[None, :]).sum(
                axis=0
            ) / nvec
            sem_acc += var.sum()

            # color: io col = lane*128 + chunk-phase
            sumI = io_d[b, 0].reshape(C, QI).sum(axis=1)
            mI = sumI / HW
            col_acc += (
                (mI[0] - mI[1]) ** 2 + (mI[0] - mI[2]) ** 2 + (mI[1] - mI[2]) ** 2
            )

            # exposure: [j, r*512+w] -> patch sums -> (Lp-E)^2
            eo = expo_d[b].reshape(K, 4, 32, PATCH).sum(-1)   # [j, r, wg]
            Lp = eo.transpose(1, 0, 2).reshape(32, 32) / (PATCH * PATCH)
            exp_acc += ((Lp - E_EXP) ** 2).sum()

            tv_acc_v += lout[b, :, 0:4].sum()
            tv_acc_h += lout[b, :, 4:8].sum()

            # band-boundary vertical diffs (3 row pairs) on host
            bsamp = core * BLOC + b
            for r in range(1, 4):
                tv_acc_v += np.abs(
                    L64[bsamp, 0, 128 * r] - L64[bsamp, 0, 128 * r - 1]
                ).sum()

    L_exp = exp_acc / (B * 32 * 32)
    L_tv = tv_acc_v / (B * 1 * (H - 1) * W) + tv_acc_h / (B * 1 * H * (W - 1))
    L_color = col_acc / B
    L_sem = sem_acc / B
    total = (
        L_EXP_W * L_exp + L_TV_W * L_tv + L_COLOR_W * L_color + L_SEM_W * L_sem
    )
    return np.float32(total)
